# revision 1
# baseline (speedup 1.0000x reference)
"""Cross-attention kernel for 8 TRN2 NeuronCores.

Reference shapes: x [4, 2048, 1024], embeds [4, 2048, 1024],
Wq/Wk/Wv [1024, 1024] (+bias), Wo [1024, 1024] (+bias), H=16 heads, D=64.

Sharding: core c handles batch b = c//2 and head group hg = c%2 (8 heads,
attn-dim slice of 512).  Each core computes a partial output
outT_c [1024, 2048] = (ctx_c @ Wo[hg-slice]) ^T; the host sums the two
partials per batch (row-parallel Wo all-reduce done at unshard time) and
adds nothing else (bo is folded into the even core's partial).

Device dataflow per core (activations kept feature-major, "T" = [feat, tok]):
  QT = Wq_c^T @ xT      [512, 2048]   (fp32r matmuls, psum fp32)
  KT = Wk_c^T @ embT    [512, 2048]
  V  = embT^T-proj      [2048, 512]   token-major, + ones column per head
  per head h, lq-half: ST = K_h @ Q_h^T  -> exp (ACT, scale=1/8) -> E
                       [C';denom] = [V_h|1]^T @ E   (ones-column trick)
                       CT_h = C' * (1/denom)  (recip + partition_broadcast)
  outT = Wo_c^T @ CT    [1024, 2048]  + bo (even core only)
Softmax skips the max-subtraction: scores ~ N(0,1), |s| < ~7, exp is safe
in fp32 and matches the reference softmax mathematically.
"""

import os
import sys

if "/opt/trn_rl_repo" not in sys.path:
    sys.path.insert(0, "/opt/trn_rl_repo")

import numpy as np

import concourse.bass as bass  # noqa: F401  (engine namespaces live on nc)
import concourse.mybir as mybir
import concourse.tile as tile
from concourse import bacc
from concourse.bass_utils import run_bass_kernel_spmd

P = 128
B, LQ, LK, DIM = 4, 2048, 2048, 1024
H, D = 16, 64
ADC = 512          # per-core attention dim (8 heads x 64)
NHC = 8            # heads per core
SCALE = 1.0 / 8.0
F32 = mybir.dt.float32
FR = mybir.dt.float32r
EXP = mybir.ActivationFunctionType.Exp

K_T = DIM // P     # 8 contraction tiles for projections
M_AD = ADC // P    # 4 ad partition tiles
T_LK = LK // P     # 16 lk tiles
VW = NHC * (D + 1)  # 520: V block width per lk tile (64 cols + ones col per head)

_CACHE = {}
_PHASES = int(os.environ.get("KPHASES", "3"))


def _build():
    nc = bacc.Bacc("TRN2", target_bir_lowering=False, debug=False)

    xT = nc.dram_tensor("xT", [DIM, LQ], FR, kind="ExternalInput").ap()
    embT = nc.dram_tensor("embT", [DIM, LK], FR, kind="ExternalInput").ap()
    Wq = nc.dram_tensor("Wq", [DIM, ADC], FR, kind="ExternalInput").ap()
    Wk = nc.dram_tensor("Wk", [DIM, ADC], FR, kind="ExternalInput").ap()
    Wv = nc.dram_tensor("Wv", [DIM, ADC], FR, kind="ExternalInput").ap()
    Wo = nc.dram_tensor("Wo", [ADC, DIM], FR, kind="ExternalInput").ap()
    bq = nc.dram_tensor("bq", [P, M_AD], F32, kind="ExternalInput").ap()
    bk = nc.dram_tensor("bk", [P, M_AD], F32, kind="ExternalInput").ap()
    bvb = nc.dram_tensor("bvb", [P, ADC], F32, kind="ExternalInput").ap()
    bo = nc.dram_tensor("bo", [P, DIM // P], F32, kind="ExternalInput").ap()
    outT = nc.dram_tensor("outT", [DIM, LQ], F32, kind="ExternalOutput").ap()

    with tile.TileContext(nc) as tc:
        with tc.tile_pool(name="resident", bufs=1) as res:
            QT = [res.tile([P, LQ], FR, name=f"qt{m}") for m in range(M_AD)]
            KT = [res.tile([P, LK], FR, name=f"kt{m}") for m in range(M_AD)]
            V = res.tile([P, T_LK * VW], FR, name="v")
            CT = [res.tile([P, LQ], FR, name=f"ct{p}") for p in range(M_AD)]
            WO = res.tile([P, ADC // P, DIM], FR, name="wo")
            bq_sb = res.tile([P, M_AD], F32, name="bq")
            bk_sb = res.tile([P, M_AD], F32, name="bk")
            bvb_sb = res.tile([P, ADC], F32, name="bvb")
            bo_sb = res.tile([P, DIM // P], F32, name="bo")

            nc.sync.dma_start(WO[:], Wo.rearrange("(k p) n -> p k n", p=P))
            nc.sync.dma_start(bq_sb[:], bq[:])
            nc.sync.dma_start(bk_sb[:], bk[:])
            nc.sync.dma_start(bvb_sb[:], bvb[:])
            nc.sync.dma_start(bo_sb[:], bo[:])

            # ones columns (col 64 of each head's 65-wide block) for the
            # fused-denominator C matmul.  memset can't write fp32r, so
            # synthesize 1.0 on DVE as in0*0 + 1.
            zsrc = res.tile([P, NHC], F32, name="zsrc")
            nc.gpsimd.memset(zsrc[:], 0.0)
            for t in range(T_LK):
                blk = V[:, t * VW:(t + 1) * VW].rearrange(
                    "p (a b) -> p a b", b=D + 1)
                nc.vector.tensor_scalar(
                    blk[:, :, D:D + 1],
                    zsrc[:].rearrange("p (a b) -> p a b", b=1),
                    0.0, 1.0,
                    op0=mybir.AluOpType.mult, op1=mybir.AluOpType.add)

            # ---------------- projections ----------------
            # V first (attention needs all of V), then Q/K pair-by-pair so
            # attention on pair 0 can start while later pairs project.
            with tc.tile_pool(name="wproj", bufs=2) as wpool, \
                 tc.tile_pool(name="stream", bufs=4) as spool, \
                 tc.tile_pool(name="pjp", bufs=1, space="PSUM") as pjp, \
                 tc.tile_pool(name="pjv", bufs=2, space="PSUM") as pjv:

                wv_sb = wpool.tile([P, K_T, ADC], FR, name="w")
                for k in range(K_T):
                    nc.sync.dma_start(wv_sb[:, k, :],
                                      Wv[k * P:(k + 1) * P, :])
                embT_kp = embT.rearrange("(k p) n -> p k n", p=P)
                for t in range(T_LK):
                    vk = spool.tile([P, K_T, P], FR, name="vk")
                    nc.sync.dma_start(
                        vk[:], embT_kp[:, :, t * P:(t + 1) * P])
                    psv = pjv.tile([P, ADC], F32, name="pv")
                    for k in range(K_T):
                        nc.tensor.matmul(psv[:], vk[:, k, :], wv_sb[:, k, :],
                                         start=(k == 0), stop=(k == K_T - 1))
                    vdst = V[:, t * VW:(t + 1) * VW].rearrange(
                        "p (a b) -> p a b", b=D + 1)[:, :, 0:D]
                    nc.vector.tensor_tensor(
                        vdst,
                        psv[:].rearrange("p (a b) -> p a b", b=D),
                        bvb_sb[:].rearrange("p (a b) -> p a b", b=D),
                        op=mybir.AluOpType.add)

                for (w_dram, b_sb, out_tiles, src) in (
                        (Wq, bq_sb, QT, xT), (Wk, bk_sb, KT, embT)):
                    w_sb = wpool.tile([P, K_T, ADC], FR, name="w")
                    for k in range(K_T):
                        nc.sync.dma_start(w_sb[:, k, :],
                                          w_dram[k * P:(k + 1) * P, :])
                    for n in range(LQ // 512):
                        pps = [pjp.tile([P, 512], F32, name=f"pp{m}")
                               for m in range(M_AD)]
                        for k in range(K_T):
                            xt = spool.tile([P, 512], FR, name="xs")
                            nc.sync.dma_start(
                                xt[:],
                                src[k * P:(k + 1) * P, n * 512:(n + 1) * 512])
                            for m in range(M_AD):
                                nc.tensor.matmul(
                                    pps[m][:],
                                    w_sb[:, k, m * P:(m + 1) * P],
                                    xt[:],
                                    start=(k == 0), stop=(k == K_T - 1))
                        for m in range(M_AD):
                            nc.vector.tensor_scalar_add(
                                out_tiles[m][:, n * 512:(n + 1) * 512],
                                pps[m][:], b_sb[:, m:m + 1])

            # ---------------- attention ----------------
            # Head pairs interleaved: the two heads of a pair occupy PE row
            # groups 0-63 / 64-127 (tile_position auto-derived from the
            # base partition), so their K=64 S-matmuls run concurrently.
            with tc.tile_pool(name="aps", bufs=1, space="PSUM") as aps, \
                 tc.tile_pool(name="apc", bufs=1, space="PSUM") as apc, \
                 tc.tile_pool(name="etp", bufs=2) as etp, \
                 tc.tile_pool(name="small", bufs=1) as small:
                for p in range(M_AD if _PHASES >= 2 else 0):
                    mt = p
                    for half in range(2):
                        q0 = half * 1024
                        pcs = [apc.tile([D + 1, 1024], F32, name=f"pc{a}")
                               for a in range(2)]
                        for t in range(T_LK):
                            pss = []
                            for a in range(2):
                                ro = a * D
                                ps = aps.tile([P, 1024], F32, name=f"ps{a}")
                                for nn in range(2):
                                    nc.tensor.matmul(
                                        ps[:, nn * 512:(nn + 1) * 512],
                                        KT[mt][ro:ro + D, t * P:(t + 1) * P],
                                        QT[mt][ro:ro + D,
                                               q0 + nn * 512:
                                               q0 + (nn + 1) * 512],
                                        start=True, stop=True)
                                pss.append(ps)
                            ets = []
                            for a in range(2):
                                et = etp.tile([P, 1024], FR, name=f"et{a}")
                                nc.scalar.activation(et[:], pss[a][:], EXP,
                                                     scale=SCALE)
                                ets.append(et)
                            for a in range(2):
                                vcol = (2 * p + a) * (D + 1)
                                for nn in range(2):
                                    nc.tensor.matmul(
                                        pcs[a][:, nn * 512:(nn + 1) * 512],
                                        V[:, t * VW + vcol:
                                           t * VW + vcol + D + 1],
                                        ets[a][:, nn * 512:(nn + 1) * 512],
                                        start=(t == 0), stop=(t == T_LK - 1))
                        for a in range(2):
                            ro = a * D
                            r1 = small.tile([1, 1024], F32, name=f"r1{a}")
                            nc.vector.reciprocal(r1[:], pcs[a][D:D + 1, :])
                            rb = small.tile([D, 1024], F32, name=f"rb{a}")
                            nc.gpsimd.partition_broadcast(rb[:], r1[0:1, :])
                            nc.vector.tensor_tensor(
                                CT[mt][ro:ro + D, q0:q0 + 1024],
                                pcs[a][0:D, :], rb[:],
                                op=mybir.AluOpType.mult)

            # ---------------- output projection ----------------
            with tc.tile_pool(name="ops", bufs=4, space="PSUM") as ops, \
                 tc.tile_pool(name="ostage", bufs=4) as ostage:
                for m in range(DIM // P if _PHASES >= 3 else 0):
                    for n in range(LQ // 512):
                        po = ops.tile([P, 512], F32, name="po")
                        for kk in range(ADC // P):
                            nc.tensor.matmul(
                                po[:],
                                WO[:, kk, m * P:(m + 1) * P],
                                CT[kk][:, n * 512:(n + 1) * 512],
                                start=(kk == 0), stop=(kk == ADC // P - 1))
                        ot = ostage.tile([P, 512], F32, name="ot")
                        nc.vector.tensor_scalar_add(ot[:], po[:],
                                                    bo_sb[:, m:m + 1])
                        nc.sync.dma_start(
                            outT[m * P:(m + 1) * P, n * 512:(n + 1) * 512],
                            ot[:])

    nc.compile()
    return nc


def _in_maps(x, embeds, Wq, bq, Wk, bk, Wv, bv, Wo, bo):
    f = np.float32
    maps = []
    for c in range(8):
        b, hg = c // 2, c % 2
        s = slice(hg * ADC, (hg + 1) * ADC)
        bo_c = bo if hg == 0 else np.zeros_like(bo)
        maps.append({
            "xT": np.ascontiguousarray(x[b].T, dtype=f),
            "embT": np.ascontiguousarray(embeds[b].T, dtype=f),
            "Wq": np.ascontiguousarray(Wq[:, s], dtype=f),
            "Wk": np.ascontiguousarray(Wk[:, s], dtype=f),
            "Wv": np.ascontiguousarray(Wv[:, s], dtype=f),
            "Wo": np.ascontiguousarray(Wo[s, :], dtype=f),
            "bq": np.ascontiguousarray(
                bq[s].reshape(M_AD, P).T, dtype=f),
            "bk": np.ascontiguousarray(
                bk[s].reshape(M_AD, P).T, dtype=f),
            "bvb": np.ascontiguousarray(
                np.tile(bv[s], (P, 1)), dtype=f),
            "bo": np.ascontiguousarray(
                bo_c.reshape(DIM // P, P).T, dtype=f),
        })
    return maps


def kernel(x, embeds, Wq, bq, Wk, bk, Wv, bv, Wo, bo, _trace=False,
           _tmpdir=None):
    x = np.asarray(x); embeds = np.asarray(embeds)
    Wq = np.asarray(Wq); bq = np.asarray(bq)
    Wk = np.asarray(Wk); bk = np.asarray(bk)
    Wv = np.asarray(Wv); bv = np.asarray(bv)
    Wo = np.asarray(Wo); bo = np.asarray(bo)

    if "nc" not in _CACHE:
        _CACHE["nc"] = _build()
    nc = _CACHE["nc"]

    maps = _in_maps(x, embeds, Wq, bq, Wk, bk, Wv, bv, Wo, bo)
    res = run_bass_kernel_spmd(nc, maps, core_ids=list(range(8)),
                               trace=_trace, tmpdir=_tmpdir)
    if _trace:
        _CACHE["last_exec_time_ns"] = res.exec_time_ns
        _CACHE["last_results"] = res

    out = np.empty((B, LQ, DIM), np.float32)
    for b in range(B):
        acc = res.results[2 * b]["outT"] + res.results[2 * b + 1]["outT"]
        out[b] = acc.T
    return out



# revision 20
# speedup vs baseline: 1.2664x; 1.2664x over previous
"""Cross-attention kernel for 8 TRN2 NeuronCores (bf16 + fp8-residual design).

Reference shapes: x [4, 2048, 1024], embeds [4, 2048, 1024],
Wq/Wk/Wv [1024, 1024] (+bias), Wo [1024, 1024] (+bias), H=16 heads, D=64.

Sharding: core c handles batch b = c//2 and head group hg = c%2 (8 heads).
Core output is the partial outT_c [1024, 2048] (bf16) = Wo[hg]^T @ ctx_c;
host sums the two partials per batch and adds bo + bv @ Wo (bv commutes
through the attention average since softmax weights sum to 1).

Numerics: diffuse attention means elementwise quantization noise passes to
the output at ~full relative strength, so activations/weights are bf16.
PE cost tricks that keep bf16-class accuracy:
  * projections: inputs as fp8e4 hi+lo residual pairs, 3 DoubleRow GEMMs
    (hi*hi + hi*lo + lo*hi) accumulated in one PSUM group -> 0.75 cyc/row
  * scores: bf16, K=64 per instr (cost only counts output free size)
  * exp: ACT native exp -> bf16 (most units) + DVE int16 Schraudolph units
    (bits16 = int16(s'' + 16000.x), E = exp(s_true/8)/4), small share so
    the log-linear sawtooth stays <0.4% overall
  * ctx token-major bf16 [128lq, 65] (ones col -> per-partition denom)
  * normalize via reciprocal + per-partition broadcast mult -> ctt bf16
  * ctt -> ctf via XBAR dma transpose (DMA engines, no ACT/DVE time)
  * out proj bf16 -> ACT copies psum -> bf16 staging -> DMA out
"""

import sys

if "/opt/trn_rl_repo" not in sys.path:
    sys.path.insert(0, "/opt/trn_rl_repo")

import numpy as np
import ml_dtypes

import concourse.bass as bass  # noqa: F401
import concourse.mybir as mybir
import concourse.tile as tile
from concourse import bacc
from concourse.bass_utils import run_bass_kernel_spmd

P = 128
B, LQ, LK, DIM = 4, 2048, 2048, 1024
H, D = 16, 64
ADC = 512            # per-core attention dim (8 heads x 64)
NHC = 8              # heads per core
F32 = mybir.dt.float32
BF16 = mybir.dt.bfloat16
FP8 = mybir.dt.float8e4
I16 = mybir.dt.int16
DR = mybir.MatmulPerfMode.DoubleRow
EXP = mybir.ActivationFunctionType.Exp
ADD = mybir.AluOpType.add
MULT = mybir.AluOpType.mult

# E stored as bf16 via bits16 = 128*log2(E) + 16256 (Schraudolph on DVE,
# exact exp on ACT).  psum scores arrive as s'' = 23.0831 * s_true
# (QK_SCALE = sqrt(128/(8 ln2)) on each of q, k), E = exp(s_true/8)/4.
QK_SCALE = 4.804530139182014       # sqrt(128 / (8 ln 2))
ACT_SCALE = 0.005415212448059204   # ln2 / 128
TRICK_B = 16000.0 - 7.33           # 16256-256, minus HW-measured sawtooth mean
ACT_BIAS = ACT_SCALE * (16000.0 - 16256.0)   # same E scale, HW-centered

_CACHE = {}

# exp-unit engine pattern per block of 8 units (A=ACT, D=DVE): ~75/25 so
# ACT stays under the PE critical path; DVE sawtooth contributes ~0.9%.
_EXP_PAT = [
    ["A", "A", "A", "D", "A", "A", "A", "D"],
    ["A", "A", "A", "D", "A", "A", "A", "D"],
]

WSCALE = 32.0   # pre-scale W for fp8 (std 0.031 -> subnormal-crushed else)


def _build():
    nc = bacc.Bacc("TRN2", target_bir_lowering=False, debug=False)

    xh = nc.dram_tensor("xh", [DIM, LQ], FP8, kind="ExternalInput").ap()
    xl = nc.dram_tensor("xl", [DIM, LQ], FP8, kind="ExternalInput").ap()
    eh = nc.dram_tensor("eh", [DIM, LK], FP8, kind="ExternalInput").ap()
    el = nc.dram_tensor("el", [DIM, LK], FP8, kind="ExternalInput").ap()
    wqh = nc.dram_tensor("wqh", [DIM, ADC], FP8, kind="ExternalInput").ap()
    wql = nc.dram_tensor("wql", [DIM, ADC], FP8, kind="ExternalInput").ap()
    wkh = nc.dram_tensor("wkh", [DIM, ADC], FP8, kind="ExternalInput").ap()
    wkl = nc.dram_tensor("wkl", [DIM, ADC], FP8, kind="ExternalInput").ap()
    wvh = nc.dram_tensor("wvh", [DIM, ADC], FP8, kind="ExternalInput").ap()
    wvl = nc.dram_tensor("wvl", [DIM, ADC], FP8, kind="ExternalInput").ap()
    WOb = nc.dram_tensor("WOb", [ADC, DIM], BF16, kind="ExternalInput").ap()
    bqp = nc.dram_tensor("bqp", [P, 4], F32, kind="ExternalInput").ap()
    bkp = nc.dram_tensor("bkp", [P, 4], F32, kind="ExternalInput").ap()
    outT = nc.dram_tensor("outT", [DIM, LQ], BF16, kind="ExternalOutput").ap()

    kp3 = lambda ap: ap.rearrange("(kp two p) m -> p kp two m", p=P, two=2)
    kt3 = lambda ap: ap.rearrange("(k p) n -> p k n", p=P)

    with tile.TileContext(nc) as tc:
        with tc.tile_pool(name="res", bufs=1) as res:
            wo_s = res.tile([P, 4, DIM], BF16, name="wo")
            bq_s = res.tile([P, 4], F32, name="bq")
            bk_s = res.tile([P, 4], F32, name="bk")
            qt = res.tile([P, 4, LQ], BF16, name="qt")
            kt = res.tile([P, 4, LK], BF16, name="kt")
            vb = res.tile([P, 16, NHC, 65], BF16, name="vb")
            ctt = res.tile([P, 16, 512], BF16, name="ctt")     # token-major
            ctf = res.tile([P, 16, 4, P], BF16, name="ctf")    # feature-major
            bias_t = res.tile([P, 1], F32, name="bias_t")

            nc.sync.dma_start(wo_s[:], kt3(WOb))
            nc.sync.dma_start(bq_s[:], bqp)
            nc.sync.dma_start(bk_s[:], bkp)
            nc.gpsimd.memset(vb[:, :, :, 64:65], 1.0)
            nc.gpsimd.memset(bias_t[:], ACT_BIAS)

            # ---------------- projections (fp8 residual pairs) -----------
            with tc.tile_pool(name="pin", bufs=1) as pin, \
                 tc.tile_pool(name="pj", bufs=4, space="PSUM") as pj:
                xh_s = pin.tile([P, 8, LQ], FP8, name="xh")
                xl_s = pin.tile([P, 8, LQ], FP8, name="xl")
                eh_s = pin.tile([P, 8, LK], FP8, name="eh")
                el_s = pin.tile([P, 8, LK], FP8, name="el")
                wq_s = pin.tile([P, 2, 4, 2, ADC], FP8, name="wq")
                wk_s = pin.tile([P, 2, 4, 2, ADC], FP8, name="wk")
                wv_s = pin.tile([P, 2, 4, 2, ADC], FP8, name="wv")
                nc.sync.dma_start(wk_s[:, 0], kp3(wkh))
                nc.sync.dma_start(wk_s[:, 1], kp3(wkl))
                nc.sync.dma_start(eh_s[:], kt3(eh))
                nc.sync.dma_start(el_s[:], kt3(el))
                nc.sync.dma_start(wq_s[:, 0], kp3(wqh))
                nc.sync.dma_start(wq_s[:, 1], kp3(wql))
                nc.sync.dma_start(xh_s[:], kt3(xh))
                nc.sync.dma_start(xl_s[:], kt3(xl))
                nc.sync.dma_start(wv_s[:, 0], kp3(wvh))
                nc.sync.dma_start(wv_s[:, 1], kp3(wvl))

                for (w_s, b_s, dst, srch, srcl) in (
                        (wk_s, bk_s, kt, eh_s, el_s),
                        (wq_s, bq_s, qt, xh_s, xl_s)):
                    for mb in range(4):
                        for n in range(4):
                            ps = pj.tile([P, 512], F32, name="pp")
                            idx = 0
                            for kp in range(4):
                                for (wi, sv) in ((0, srch), (1, srch),
                                                 (0, srcl)):
                                    nc.tensor.matmul(
                                        ps[:],
                                        w_s[:, wi, kp, :,
                                            mb * P:(mb + 1) * P],
                                        sv[:, 2 * kp:2 * kp + 2,
                                           n * 512:(n + 1) * 512],
                                        perf_mode=DR,
                                        start=(idx == 0), stop=(idx == 11))
                                    idx += 1
                            nc.vector.tensor_scalar(
                                dst[:, mb, n * 512:(n + 1) * 512],
                                ps[:], b_s[:, mb:mb + 1], QK_SCALE / WSCALE,
                                op0=ADD, op1=MULT)
                for t in range(16):
                    ps = pj.tile([P, 512], F32, name="pv")
                    idx = 0
                    for kp in range(4):
                        for (wi, sv) in ((0, eh_s), (1, eh_s), (0, el_s)):
                            nc.tensor.matmul(
                                ps[:],
                                sv[:, 2 * kp:2 * kp + 2, t * P:(t + 1) * P],
                                wv_s[:, wi, kp, :, :],
                                perf_mode=DR,
                                start=(idx == 0), stop=(idx == 11))
                            idx += 1
                    nc.vector.tensor_scalar(
                        vb[:, t, :, 0:64],
                        ps[:].rearrange("p (h d) -> p h d", d=64),
                        1.0 / WSCALE, None, op0=MULT)

            # ---------------- attention + output ----------------
            with tc.tile_pool(name="sps", bufs=2, space="PSUM") as sps, \
                 tc.tile_pool(name="cps", bufs=2, space="PSUM") as cps, \
                 tc.tile_pool(name="ops", bufs=2, space="PSUM") as ops, \
                 tc.tile_pool(name="e8p", bufs=2) as e8p, \
                 tc.tile_pool(name="rcp", bufs=2) as rcp, \
                 tc.tile_pool(name="osg", bufs=2) as osg:

                def scores_and_exp(h, n, bi):
                    g2, a = h // 2, h % 2
                    eb = e8p.tile([P, 16, 512], BF16, name="eb")
                    pat = _EXP_PAT[bi % 2]
                    for u in range(8):
                        sp = sps.tile([P, 1024], F32, name="sp")
                        for i in range(2):
                            t = 2 * u + i
                            nc.tensor.matmul(
                                sp[:, i * 512:(i + 1) * 512],
                                kt[64 * a:64 * a + 64, g2,
                                   t * P:(t + 1) * P],
                                qt[64 * a:64 * a + 64, g2,
                                   n * 512:(n + 1) * 512],
                                start=True, stop=True,
                                tile_position=(64 * a, 0))
                        dst = eb[:, 2 * u:2 * u + 2, :]
                        if pat[u] == "A":
                            nc.scalar.activation(dst, sp[:], EXP,
                                                 scale=ACT_SCALE,
                                                 bias=bias_t[:])
                        else:
                            nc.vector.tensor_scalar(
                                dst.bitcast(I16), sp[:], TRICK_B, None,
                                op0=ADD)
                    return eb

                def ctx_block(h, n, eb):
                    cp = cps.tile([P, 4, 65], F32, name="cp")
                    for j in range(4):
                        for t in range(16):
                            nc.tensor.matmul(
                                cp[:, j, :],
                                eb[:, t, j * P:(j + 1) * P],
                                vb[:, t, h, :],
                                start=(t == 0), stop=(t == 15))
                    return cp

                def norm_block(h, n, cp):
                    rc = rcp.tile([P, 4], F32, name="rc")
                    nc.vector.reciprocal(rc[:], cp[:, :, 64])
                    nc.vector.tensor_tensor(
                        ctt[:, 4 * n:4 * n + 4, 64 * h:64 * h + 64],
                        cp[:, :, 0:64],
                        rc[:].to_broadcast((P, 4, 64)),
                        op=MULT)

                def transpose_col(n):
                    for j in range(4):
                        nc.sync.dma_start_transpose(
                            ctf[:, 4 * n + j], ctt[:, 4 * n + j, :])

                def outproj_col(n, m2):
                    og = osg.tile([P, 2, 512], BF16, name="og")
                    for i in range(2):
                        m = 2 * m2 + i
                        po = ops.tile([P, 512], F32, name="po", tag="po")
                        for kk in range(4):
                            nc.tensor.matmul(
                                po[:],
                                wo_s[:, kk, m * P:(m + 1) * P],
                                ctf[:, 4 * n:4 * n + 4, kk, :],
                                start=(kk == 0), stop=(kk == 3))
                        nc.vector.tensor_copy(og[:, i, :], po[:])
                    nc.sync.dma_start(
                        outT.rearrange("(mm p) n -> p mm n", p=P)
                        [:, 2 * m2:2 * m2 + 2, n * 512:(n + 1) * 512],
                        og[:])

                bi = 0
                for n in range(4):
                    prev = None
                    for h in range(8):
                        eb = scores_and_exp(h, n, bi)
                        bi += 1
                        if prev is not None:
                            norm_block(prev[0], n, ctx_block(prev[0], n,
                                                            prev[1]))
                        if n > 0 and h >= 4:
                            outproj_col(n - 1, h - 4)
                        prev = (h, eb)
                    norm_block(prev[0], n, ctx_block(prev[0], n, prev[1]))
                    transpose_col(n)
                for m2 in range(4):
                    outproj_col(3, m2)

    nc.compile()
    return nc


def _pair(a, f8):
    hi = a.astype(f8)
    lo = (a.astype(np.float32) - hi.astype(np.float32)).astype(f8)
    return hi, lo


def _in_maps(x, embeds, Wq, bq, Wk, bk, Wv, Wo):
    f8 = ml_dtypes.float8_e4m3
    bf = ml_dtypes.bfloat16
    maps = []
    for c in range(8):
        b, hg = c // 2, c % 2
        s = slice(hg * ADC, (hg + 1) * ADC)
        xh_, xl_ = _pair(np.ascontiguousarray(x[b].T), f8)
        eh_, el_ = _pair(np.ascontiguousarray(embeds[b].T), f8)
        wqh_, wql_ = _pair(np.ascontiguousarray(Wq[:, s]) * WSCALE, f8)
        wkh_, wkl_ = _pair(np.ascontiguousarray(Wk[:, s]) * WSCALE, f8)
        wvh_, wvl_ = _pair(np.ascontiguousarray(Wv[:, s]) * WSCALE, f8)
        maps.append({
            "xh": xh_, "xl": xl_, "eh": eh_, "el": el_,
            "wqh": wqh_, "wql": wql_, "wkh": wkh_, "wkl": wkl_,
            "wvh": wvh_, "wvl": wvl_,
            "WOb": np.ascontiguousarray(Wo[s, :]).astype(bf),
            "bqp": np.ascontiguousarray(bq[s].reshape(4, P).T * WSCALE,
                                        dtype=np.float32),
            "bkp": np.ascontiguousarray(bk[s].reshape(4, P).T * WSCALE,
                                        dtype=np.float32),
        })
    return maps


def kernel(x, embeds, Wq, bq, Wk, bk, Wv, bv, Wo, bo, _trace=False,
           _tmpdir=None):
    x = np.asarray(x); embeds = np.asarray(embeds)
    Wq = np.asarray(Wq); bq = np.asarray(bq)
    Wk = np.asarray(Wk); bk = np.asarray(bk)
    Wv = np.asarray(Wv); bv = np.asarray(bv)
    Wo = np.asarray(Wo); bo = np.asarray(bo)

    if "nc" not in _CACHE:
        _CACHE["nc"] = _build()
    nc = _CACHE["nc"]

    maps = _in_maps(x, embeds, Wq, bq, Wk, bk, Wv, Wo)
    res = run_bass_kernel_spmd(nc, maps, core_ids=list(range(8)),
                               trace=_trace, tmpdir=_tmpdir)
    if _trace:
        _CACHE["last_exec_time_ns"] = res.exec_time_ns
        _CACHE["last_results"] = res

    out_bias = (bv.astype(np.float64) @ Wo.astype(np.float64)
                + bo.astype(np.float64)).astype(np.float32)
    out = np.empty((B, LQ, DIM), np.float32)
    for b in range(B):
        acc = (res.results[2 * b]["outT"].astype(np.float32)
               + res.results[2 * b + 1]["outT"].astype(np.float32))
        out[b] = acc.T + out_bias
    return out


# revision 21
# speedup vs baseline: 1.2901x; 1.0187x over previous
"""Cross-attention kernel for 8 TRN2 NeuronCores (bf16 + fp8-residual design).

Reference shapes: x [4, 2048, 1024], embeds [4, 2048, 1024],
Wq/Wk/Wv [1024, 1024] (+bias), Wo [1024, 1024] (+bias), H=16 heads, D=64.

Sharding: core c handles batch b = c//2 and head group hg = c%2 (8 heads).
Core output is the partial outT_c [1024, 2048] (bf16) = Wo[hg]^T @ ctx_c;
host sums the two partials per batch and adds bo + bv @ Wo (bv commutes
through the attention average since softmax weights sum to 1).

Numerics: diffuse attention means elementwise quantization noise passes to
the output at ~full relative strength, so activations/weights are bf16.
PE cost tricks that keep bf16-class accuracy:
  * projections: inputs as fp8e4 hi+lo residual pairs, 3 DoubleRow GEMMs
    (hi*hi + hi*lo + lo*hi) accumulated in one PSUM group -> 0.75 cyc/row
  * scores: bf16, K=64 per instr (cost only counts output free size)
  * exp: ACT native exp -> bf16 (most units) + DVE int16 Schraudolph units
    (bits16 = int16(s'' + 16000.x), E = exp(s_true/8)/4), small share so
    the log-linear sawtooth stays <0.4% overall
  * ctx token-major bf16 [128lq, 65] (ones col -> per-partition denom)
  * normalize via reciprocal + per-partition broadcast mult -> ctt bf16
  * ctt -> ctf via XBAR dma transpose (DMA engines, no ACT/DVE time)
  * out proj bf16 -> ACT copies psum -> bf16 staging -> DMA out
"""

import sys

if "/opt/trn_rl_repo" not in sys.path:
    sys.path.insert(0, "/opt/trn_rl_repo")

import numpy as np
import ml_dtypes

import concourse.bass as bass  # noqa: F401
import concourse.mybir as mybir
import concourse.tile as tile
from concourse import bacc
from concourse.bass_utils import run_bass_kernel_spmd

P = 128
B, LQ, LK, DIM = 4, 2048, 2048, 1024
H, D = 16, 64
ADC = 512            # per-core attention dim (8 heads x 64)
NHC = 8              # heads per core
F32 = mybir.dt.float32
BF16 = mybir.dt.bfloat16
FP8 = mybir.dt.float8e4
I16 = mybir.dt.int16
DR = mybir.MatmulPerfMode.DoubleRow
EXP = mybir.ActivationFunctionType.Exp
ADD = mybir.AluOpType.add
MULT = mybir.AluOpType.mult

# E stored as bf16 via bits16 = 128*log2(E) + 16256 (Schraudolph on DVE,
# exact exp on ACT).  psum scores arrive as s'' = 23.0831 * s_true
# (QK_SCALE = sqrt(128/(8 ln2)) on each of q, k), E = exp(s_true/8)/4.
QK_SCALE = 4.804530139182014       # sqrt(128 / (8 ln 2))
ACT_SCALE = 0.005415212448059204   # ln2 / 128
TRICK_B = 16000.0 - 7.33           # 16256-256, minus HW-measured sawtooth mean
ACT_BIAS = ACT_SCALE * (16000.0 - 16256.0)   # same E scale, HW-centered

_CACHE = {}

# exp-unit engine pattern per block of 8 units (A=ACT, D=DVE): ~75/25 so
# ACT stays under the PE critical path; DVE sawtooth contributes ~0.9%.
_EXP_PAT = [
    ["A", "A", "A", "D", "A", "A", "A", "D"],
    ["A", "A", "A", "D", "A", "A", "A", "D"],
]

WSCALE = 32.0   # pre-scale W for fp8 (std 0.031 -> subnormal-crushed else)


def _build():
    nc = bacc.Bacc("TRN2", target_bir_lowering=False, debug=False)

    xh = nc.dram_tensor("xh", [DIM, LQ], FP8, kind="ExternalInput").ap()
    xl = nc.dram_tensor("xl", [DIM, LQ], FP8, kind="ExternalInput").ap()
    eh = nc.dram_tensor("eh", [DIM, LK], FP8, kind="ExternalInput").ap()
    el = nc.dram_tensor("el", [DIM, LK], FP8, kind="ExternalInput").ap()
    wqh = nc.dram_tensor("wqh", [DIM, ADC], FP8, kind="ExternalInput").ap()
    wql = nc.dram_tensor("wql", [DIM, ADC], FP8, kind="ExternalInput").ap()
    wkh = nc.dram_tensor("wkh", [DIM, ADC], FP8, kind="ExternalInput").ap()
    wkl = nc.dram_tensor("wkl", [DIM, ADC], FP8, kind="ExternalInput").ap()
    wvh = nc.dram_tensor("wvh", [DIM, ADC], FP8, kind="ExternalInput").ap()
    wvl = nc.dram_tensor("wvl", [DIM, ADC], FP8, kind="ExternalInput").ap()
    WOb = nc.dram_tensor("WOb", [ADC, DIM], BF16, kind="ExternalInput").ap()
    bqp = nc.dram_tensor("bqp", [P, 4], F32, kind="ExternalInput").ap()
    bkp = nc.dram_tensor("bkp", [P, 4], F32, kind="ExternalInput").ap()
    outT = nc.dram_tensor("outT", [DIM, LQ], BF16, kind="ExternalOutput").ap()

    kp3 = lambda ap: ap.rearrange("(kp two p) m -> p kp two m", p=P, two=2)
    kt3 = lambda ap: ap.rearrange("(k p) n -> p k n", p=P)

    with tile.TileContext(nc) as tc:
        with tc.tile_pool(name="res", bufs=1) as res:
            wo_s = res.tile([P, 4, DIM], BF16, name="wo")
            bq_s = res.tile([P, 4], F32, name="bq")
            bk_s = res.tile([P, 4], F32, name="bk")
            qt = res.tile([P, 4, LQ], BF16, name="qt")
            kt = res.tile([P, 4, LK], BF16, name="kt")
            vb = res.tile([P, 16, NHC, 65], BF16, name="vb")
            ctt = res.tile([P, 16, 512], BF16, name="ctt")     # token-major
            ctf = res.tile([P, 16, 4, P], BF16, name="ctf")    # feature-major
            bias_t = res.tile([P, 1], F32, name="bias_t")

            nc.sync.dma_start(wo_s[:], kt3(WOb))
            nc.sync.dma_start(bq_s[:], bqp)
            nc.sync.dma_start(bk_s[:], bkp)
            nc.gpsimd.memset(vb[:, :, :, 64:65], 1.0)
            nc.gpsimd.memset(bias_t[:], ACT_BIAS)

            # ---------------- projections (fp8 residual pairs) -----------
            with tc.tile_pool(name="pin", bufs=1) as pin, \
                 tc.tile_pool(name="pj", bufs=4, space="PSUM") as pj:
                xh_s = pin.tile([P, 8, LQ], FP8, name="xh")
                xl_s = pin.tile([P, 8, LQ], FP8, name="xl")
                eh_s = pin.tile([P, 8, LK], FP8, name="eh")
                el_s = pin.tile([P, 8, LK], FP8, name="el")
                wq_s = pin.tile([P, 2, 4, 2, ADC], FP8, name="wq")
                wk_s = pin.tile([P, 2, 4, 2, ADC], FP8, name="wk")
                wv_s = pin.tile([P, 2, 4, 2, ADC], FP8, name="wv")
                nc.sync.dma_start(wk_s[:, 0], kp3(wkh))
                nc.sync.dma_start(wk_s[:, 1], kp3(wkl))
                nc.sync.dma_start(eh_s[:], kt3(eh))
                nc.sync.dma_start(el_s[:], kt3(el))
                nc.sync.dma_start(wq_s[:, 0], kp3(wqh))
                nc.sync.dma_start(wq_s[:, 1], kp3(wql))
                nc.sync.dma_start(xh_s[:], kt3(xh))
                nc.sync.dma_start(xl_s[:], kt3(xl))
                nc.sync.dma_start(wv_s[:, 0], kp3(wvh))
                nc.sync.dma_start(wv_s[:, 1], kp3(wvl))

                for (w_s, b_s, dst, srch, srcl) in (
                        (wk_s, bk_s, kt, eh_s, el_s),
                        (wq_s, bq_s, qt, xh_s, xl_s)):
                    for mb in range(4):
                        for n in range(4):
                            ps = pj.tile([P, 512], F32, name="pp")
                            idx = 0
                            for kp in range(4):
                                for (wi, sv) in ((0, srch), (1, srch),
                                                 (0, srcl)):
                                    nc.tensor.matmul(
                                        ps[:],
                                        w_s[:, wi, kp, :,
                                            mb * P:(mb + 1) * P],
                                        sv[:, 2 * kp:2 * kp + 2,
                                           n * 512:(n + 1) * 512],
                                        perf_mode=DR,
                                        start=(idx == 0), stop=(idx == 11))
                                    idx += 1
                            nc.vector.tensor_scalar(
                                dst[:, mb, n * 512:(n + 1) * 512],
                                ps[:], b_s[:, mb:mb + 1], QK_SCALE / WSCALE,
                                op0=ADD, op1=MULT)
                for t in range(16):
                    ps = pj.tile([P, 512], F32, name="pv")
                    idx = 0
                    for kp in range(4):
                        for (wi, sv) in ((0, eh_s), (1, eh_s), (0, el_s)):
                            nc.tensor.matmul(
                                ps[:],
                                sv[:, 2 * kp:2 * kp + 2, t * P:(t + 1) * P],
                                wv_s[:, wi, kp, :, :],
                                perf_mode=DR,
                                start=(idx == 0), stop=(idx == 11))
                            idx += 1
                    nc.vector.tensor_scalar(
                        vb[:, t, :, 0:64],
                        ps[:].rearrange("p (h d) -> p h d", d=64),
                        1.0 / WSCALE, None, op0=MULT)

            # ---------------- attention + output ----------------
            with tc.tile_pool(name="sps", bufs=2, space="PSUM") as sps, \
                 tc.tile_pool(name="cps", bufs=2, space="PSUM") as cps, \
                 tc.tile_pool(name="ops", bufs=2, space="PSUM") as ops, \
                 tc.tile_pool(name="e8p", bufs=2) as e8p, \
                 tc.tile_pool(name="rcp", bufs=2) as rcp, \
                 tc.tile_pool(name="osg", bufs=2) as osg:

                def ctx_mm(cp, eb, h, j, t):
                    nc.tensor.matmul(
                        cp[:, j, :],
                        eb[:, t, j * P:(j + 1) * P],
                        vb[:, t, h, :],
                        start=(t == 0), stop=(t == 15))

                def norm_block(h, n, cp):
                    rc = rcp.tile([P, 4], F32, name="rc")
                    nc.vector.reciprocal(rc[:], cp[:, :, 64])
                    nc.vector.tensor_tensor(
                        ctt[:, 4 * n:4 * n + 4, 64 * h:64 * h + 64],
                        cp[:, :, 0:64],
                        rc[:].to_broadcast((P, 4, 64)),
                        op=MULT)

                def transpose_col(n):
                    for j in range(4):
                        nc.sync.dma_start_transpose(
                            ctf[:, 4 * n + j], ctt[:, 4 * n + j, :])

                def outproj_col(n, m2):
                    og = osg.tile([P, 2, 512], BF16, name="og")
                    for i in range(2):
                        m = 2 * m2 + i
                        po = ops.tile([P, 512], F32, name="po", tag="po")
                        for kk in range(4):
                            nc.tensor.matmul(
                                po[:],
                                wo_s[:, kk, m * P:(m + 1) * P],
                                ctf[:, 4 * n:4 * n + 4, kk, :],
                                start=(kk == 0), stop=(kk == 3))
                        nc.vector.tensor_copy(og[:, i, :], po[:])
                    nc.sync.dma_start(
                        outT.rearrange("(mm p) n -> p mm n", p=P)
                        [:, 2 * m2:2 * m2 + 2, n * 512:(n + 1) * 512],
                        og[:])

                def emit_block(h, n, bi, prev, opj):
                    """Scores+exp for (h,n); prev block's ctx instrs
                    interleaved between score units so PE fills exp-wait
                    gaps; prev's norm (and transpose when prev closes a
                    column) at the end; one outproj piece mid-block."""
                    g2, a = h // 2, h % 2
                    eb = e8p.tile([P, 16, 512], BF16, name="eb")
                    pat = _EXP_PAT[bi % 2]
                    if prev is not None:
                        ph, pn, peb = prev
                        cp = cps.tile([P, 4, 65], F32, name="cp")
                    ci = 0
                    for u in range(8):
                        sp = sps.tile([P, 1024], F32, name="sp")
                        for i in range(2):
                            t = 2 * u + i
                            nc.tensor.matmul(
                                sp[:, i * 512:(i + 1) * 512],
                                kt[64 * a:64 * a + 64, g2,
                                   t * P:(t + 1) * P],
                                qt[64 * a:64 * a + 64, g2,
                                   n * 512:(n + 1) * 512],
                                start=True, stop=True,
                                tile_position=(64 * a, 0))
                        dst = eb[:, 2 * u:2 * u + 2, :]
                        if pat[u] == "A":
                            nc.scalar.activation(dst, sp[:], EXP,
                                                 scale=ACT_SCALE,
                                                 bias=bias_t[:])
                        else:
                            nc.vector.tensor_scalar(
                                dst.bitcast(I16), sp[:], TRICK_B, None,
                                op0=ADD)
                        if prev is not None and u >= 2:
                            tgt = (u - 1) * 64 // 6
                            while ci < tgt:
                                ctx_mm(cp, peb, ph, ci // 16, ci % 16)
                                ci += 1
                        if opj is not None and u == 4:
                            outproj_col(*opj)
                    if prev is not None:
                        while ci < 64:
                            ctx_mm(cp, peb, ph, ci // 16, ci % 16)
                            ci += 1
                        norm_block(ph, pn, cp)
                        if ph == 7:
                            transpose_col(pn)
                    return eb

                bi = 0
                prev = None
                for n in range(4):
                    for h in range(8):
                        opj = (n - 1, h - 4) if (n > 0 and h >= 4) else None
                        eb = emit_block(h, n, bi, prev, opj)
                        bi += 1
                        prev = (h, n, eb)
                # tail: last block's ctx/norm, last column transpose+outproj
                cp = cps.tile([P, 4, 65], F32, name="cp")
                for ci in range(64):
                    ctx_mm(cp, prev[2], 7, ci // 16, ci % 16)
                norm_block(7, 3, cp)
                transpose_col(3)
                for m2 in range(4):
                    outproj_col(3, m2)

    nc.compile()
    return nc


def _pair(a, f8):
    hi = a.astype(f8)
    lo = (a.astype(np.float32) - hi.astype(np.float32)).astype(f8)
    return hi, lo


def _in_maps(x, embeds, Wq, bq, Wk, bk, Wv, Wo):
    f8 = ml_dtypes.float8_e4m3
    bf = ml_dtypes.bfloat16
    maps = []
    for c in range(8):
        b, hg = c // 2, c % 2
        s = slice(hg * ADC, (hg + 1) * ADC)
        xh_, xl_ = _pair(np.ascontiguousarray(x[b].T), f8)
        eh_, el_ = _pair(np.ascontiguousarray(embeds[b].T), f8)
        wqh_, wql_ = _pair(np.ascontiguousarray(Wq[:, s]) * WSCALE, f8)
        wkh_, wkl_ = _pair(np.ascontiguousarray(Wk[:, s]) * WSCALE, f8)
        wvh_, wvl_ = _pair(np.ascontiguousarray(Wv[:, s]) * WSCALE, f8)
        maps.append({
            "xh": xh_, "xl": xl_, "eh": eh_, "el": el_,
            "wqh": wqh_, "wql": wql_, "wkh": wkh_, "wkl": wkl_,
            "wvh": wvh_, "wvl": wvl_,
            "WOb": np.ascontiguousarray(Wo[s, :]).astype(bf),
            "bqp": np.ascontiguousarray(bq[s].reshape(4, P).T * WSCALE,
                                        dtype=np.float32),
            "bkp": np.ascontiguousarray(bk[s].reshape(4, P).T * WSCALE,
                                        dtype=np.float32),
        })
    return maps


def kernel(x, embeds, Wq, bq, Wk, bk, Wv, bv, Wo, bo, _trace=False,
           _tmpdir=None):
    x = np.asarray(x); embeds = np.asarray(embeds)
    Wq = np.asarray(Wq); bq = np.asarray(bq)
    Wk = np.asarray(Wk); bk = np.asarray(bk)
    Wv = np.asarray(Wv); bv = np.asarray(bv)
    Wo = np.asarray(Wo); bo = np.asarray(bo)

    if "nc" not in _CACHE:
        _CACHE["nc"] = _build()
    nc = _CACHE["nc"]

    maps = _in_maps(x, embeds, Wq, bq, Wk, bk, Wv, Wo)
    res = run_bass_kernel_spmd(nc, maps, core_ids=list(range(8)),
                               trace=_trace, tmpdir=_tmpdir)
    if _trace:
        _CACHE["last_exec_time_ns"] = res.exec_time_ns
        _CACHE["last_results"] = res

    out_bias = (bv.astype(np.float64) @ Wo.astype(np.float64)
                + bo.astype(np.float64)).astype(np.float32)
    out = np.empty((B, LQ, DIM), np.float32)
    for b in range(B):
        acc = (res.results[2 * b]["outT"].astype(np.float32)
               + res.results[2 * b + 1]["outT"].astype(np.float32))
        out[b] = acc.T + out_bias
    return out
